# revision 23
# baseline (speedup 1.0000x reference)
"""Multi-head attention on 8 Trainium2 NeuronCores (v2).

Problem: x[2, 2048, 1024] -> qkv proj (w_qkv [1024, 3072], 16 heads x 64) ->
softmax attention -> out proj (w_out [1024, 1024] + b_out).

Sharding: core c in 0..7 handles batch b = c // 4 and heads 4*(c%4) .. 4*(c%4)+3.
Each core computes a partial output projection over its 4 heads' slice; the four
cores of each batch group ReduceScatter(add) the partials chunk-by-chunk in bf16
directly into the output DRAM tensor (bias folded into the projection on the
group-leader core only). Core g of a group ends up with rows
[start + g*(len/4), +len/4) of each chunk; the host reassembles.

v2 changes vs v1 (421-465us baseline):
  - x / w_qkv / w_v shipped as bf16 (half the DMA, same PE rate), packed so
    every DMA is one contiguous 64-256KB block.
  - attention starts after only k01/q01/v are projected; the q23/k23 qkT
    chains run as PE filler inside the first attention block, so the PE
    never idles long enough for HAM to re-throttle the clock.
  - softmax denominator: reciprocal_approx_fast (DVE, ~5x faster than
    bit-exact reciprocal) + GpSimd partition_broadcast (replaces a PE K=1
    broadcast matmul + DVE copy); the normalize multiply reads the attnV
    PSUM accumulator directly (no staging copy).
  - bias folded into the out-projection via a ones-row in o_sb (partition 64)
    and a bias row in wo_sb; eviction is a plain f32->bf16 cast.
  - ReduceScatter writes straight into the y output DRAM tensor (no
    SBUF round-trip), last chunk split 2x256 rows to shrink the tail.
"""

import numpy as np

N = 2048          # sequence length per batch
D = 1024          # model dim
DH = 64           # head dim
HPC = 4           # heads per core
NCORES = 8
GSIZE = 4         # cores per reduce group
SCALE = DH ** -0.5
KB = D // 128     # 8 contraction blocks for the projections
JB = N // 128     # 16 key blocks
NCH = N // 512    # 512-query attention blocks per pair
VW = DH + 1       # v columns per head incl. ones column
NG = 8            # key groups (2 jb each) per attention block

# (row0, nrows, out_row0) of each ReduceScatter chunk
CHUNKS = [(0, 512, 0), (512, 512, 128), (1024, 512, 256),
          (1536, 256, 384), (1792, 256, 448)]

_cached = {}


def _build_nc():
    from contextlib import ExitStack

    import concourse.bacc as bacc
    import concourse.mybir as mybir
    from concourse import tile

    f32 = mybir.dt.float32
    f32r = mybir.dt.float32r
    bf16 = mybir.dt.bfloat16

    nc = bacc.Bacc(num_devices=NCORES)

    # xT packed on host as [q, kb, 128, 512] -> [4096, 512] so each (q, kb)
    # chunk is one contiguous 128KB DMA.
    xT4 = nc.declare_dram_parameter("xT4", [4 * KB * 128, 512], bf16, isOutput=False)
    wqk = nc.declare_dram_parameter("wqk", [D, 2 * HPC * DH], bf16, isOutput=False)
    wv = nc.declare_dram_parameter("wv", [D, HPC * DH], bf16, isOutput=False)
    wout = nc.declare_dram_parameter("wout", [HPC * DH, D], bf16, isOutput=False)
    bias = nc.declare_dram_parameter("bias", [1, D], bf16, isOutput=False)
    # per-core output: this core's ReduceScatter shards, bf16
    y_out = nc.declare_dram_parameter("y", [512, D], bf16, isOutput=True)

    groups = [[0, 1, 2, 3], [4, 5, 6, 7]]

    with tile.TileContext(nc) as tc:
        ctx = ExitStack()
        with ctx:
            sb = ctx.enter_context(tc.tile_pool(name="sb", bufs=1))
            sb_attn = ctx.enter_context(tc.tile_pool(name="sb_attn", bufs=12))
            sb_work = ctx.enter_context(tc.tile_pool(name="sb_work", bufs=4))
            sb_nrm = ctx.enter_context(tc.tile_pool(name="sb_nrm", bufs=4))
            # one shared 3-buffer pool (4KB buffers = 6 banks) for st/yps/mps:
            # st gets 3-deep rotation outside projection windows, so the S
            # matmuls don't ping-pong with the ACT exp drain
            ps_big = ctx.enter_context(tc.tile_pool(name="ps_big", bufs=3, space="PSUM"))
            ps_o = ctx.enter_context(tc.tile_pool(name="ps_o", bufs=2, space="PSUM"))
            dram = ctx.enter_context(tc.tile_pool(name="dram", bufs=1, space="DRAM"))

            # persistent SBUF residents
            qk_sb = sb.tile([128, 4, N], bf16, tag="qk")      # mb: q01|q23|k01|k23
            v_sb = sb.tile([128, JB, HPC * VW], bf16, tag="v")
            # normalized attn out, head-pair packed: pair p's even head on
            # partitions 0-63, odd head on 64-127 -> K=128 out-proj matmuls
            o_sb = sb.tile([128, 2, N], bf16, tag="o")
            wo_sb = sb.tile([128, 2, D], bf16, tag="wo")
            bias_sb = sb.tile([1, D], bf16, tag="bias")
            bias_bc = sb.tile([128, D], bf16, tag="bias_bc")
            scr = sb.tile([1, 16], f32, tag="scr")

            for p in range(2):
                for s in range(2):
                    h = 2 * p + s
                    nc.sync.dma_start(out=wo_sb[s * 64:(s + 1) * 64, p, :],
                                      in_=wout[h * DH:(h + 1) * DH, :])
            nc.sync.dma_start(out=bias_sb[:], in_=bias[:, :])
            # force the exp table set to load during the projection phase
            nc.scalar.activation(scr[:], bias_sb[0:1, 0:16],
                                 mybir.ActivationFunctionType.Exp, scale=0.0)
            nc.gpsimd.partition_broadcast(bias_bc[:], bias_sb[:])
            nc.vector.memset(v_sb[:], 1.0)

            y_part = dram.tile([N, D], bf16, tag="y_part")
            y_red = dram.tile([512, D], bf16, tag="y_red")

            # ---- stage 1 tiles (xT + proj weights; alive through block b0) ----
            s1 = ExitStack()
            sb_x = s1.enter_context(tc.tile_pool(name="sb_x", bufs=1))
            xT_sb = s1.enter_context(tc.tile_pool(name="sb_xT", bufs=1)) \
                .tile([128, KB, N], bf16, tag="xT")
            wqk_sb = sb_x.tile([128, KB, 2 * HPC * DH], bf16, tag="wqk")
            wv_sb = sb_x.tile([128, KB, HPC * DH], bf16, tag="wv")

            for q in range(4):
                for kb in range(KB):
                    if q == 0:  # pair each x chunk with the wqk slice it needs
                        nc.sync.dma_start(out=wqk_sb[:, kb, :],
                                          in_=wqk[kb * 128:(kb + 1) * 128, :])
                    nc.sync.dma_start(
                        out=xT_sb[:, kb, q * 512:(q + 1) * 512],
                        in_=xT4[(q * KB + kb) * 128:(q * KB + kb + 1) * 128, :])
                if q == 0:  # v weights needed only by the first v_chain
                    for kb in range(KB):
                        nc.sync.dma_start(out=wv_sb[:, kb, :],
                                          in_=wv[kb * 128:(kb + 1) * 128, :])

            def qkT_chain(mb, ich):
                """qk_sb[:, mb, ich*512:+512] = wqk[mb].T @ x[ich cols]"""
                mps = ps_big.tile([128, 512], f32, tag="big", name=f"mps{mb}_{ich}")
                for kb in range(KB):
                    nc.tensor.matmul(
                        mps[:],
                        wqk_sb[:, kb, mb * 128:(mb + 1) * 128],
                        xT_sb[:, kb, ich * 512:(ich + 1) * 512],
                        start=(kb == 0), stop=(kb == KB - 1))
                nc.vector.tensor_copy(qk_sb[:, mb, ich * 512:(ich + 1) * 512], mps[:])

            def v_chain(jb):
                vps = ps_big.tile([128, 256], f32, tag="big", name=f"vps{jb}")
                for kb in range(KB):
                    nc.tensor.matmul(
                        vps[:],
                        xT_sb[:, kb, jb * 128:(jb + 1) * 128],
                        wv_sb[:, kb, :],
                        start=(kb == 0), stop=(kb == KB - 1))
                nc.vector.tensor_copy(
                    v_sb[:, jb, :].rearrange("p (h c) -> p h c", c=VW)[:, :, 0:DH],
                    vps[:].rearrange("p (h c) -> p h c", c=DH))

            # ---- attention ----
            def attention_block(ich, pair, avb, inserts, pre_g=None):
                isl = slice(ich * 512, (ich + 1) * 512)
                ops = {}
                for s in range(2):
                    h = pair * 2 + s
                    ops[s] = ps_o.tile([65, 512], f32, tag="o", name=f"ops{h}_{ich}")
                ats = {}
                slot = 0
                for g in range(NG):
                    if pre_g:
                        for fn in pre_g.get(g, []):
                            fn()
                    for s in range(2):
                        h = pair * 2 + s
                        psl = slice(s * 64, s * 64 + 64)
                        st = ps_big.tile([128, 1024], f32, tag="big", name=f"st{h}_{ich}_{g}")
                        for u in range(2):
                            jb = 2 * g + u
                            nc.tensor.matmul(
                                st[:, u * 512:(u + 1) * 512],
                                qk_sb[psl, 2 + pair, jb * 128:(jb + 1) * 128],
                                qk_sb[psl, pair, isl],
                                start=True, stop=True)
                        at = sb_attn.tile([128, 1024], bf16, tag="attn",
                                          name=f"at{h}_{ich}_{g}")
                        nc.scalar.activation(at[:], st[:],
                                             mybir.ActivationFunctionType.Exp,
                                             scale=float(SCALE))
                        ats[s, g] = at
                    if g % avb == avb - 1:
                        for s in range(2):
                            h = pair * 2 + s
                            for gg in range(g - avb + 1, g + 1):
                                for u in range(2):
                                    jb = 2 * gg + u
                                    nc.tensor.matmul(
                                        ops[s][:],
                                        v_sb[:, jb, h * VW:(h + 1) * VW],
                                        ats[s, gg][:, u * 512:(u + 1) * 512],
                                        start=(jb == 0), stop=(jb == JB - 1))
                        for fn in inserts.get(slot, []):
                            fn()
                        slot += 1
                # inline normalize: 1/denom broadcast, numerator straight from PSUM
                for s in range(2):
                    h = pair * 2 + s
                    # custom DVE ops ignore base_partition: stage the denom row
                    # (PSUM partition 64) at partition 0 before the fast recip
                    den = sb_nrm.tile([1, 512], f32, tag="den", name=f"den{h}_{ich}")
                    nc.vector.tensor_copy(den[:], ops[s][64:65, :])
                    rden = sb_nrm.tile([1, 512], f32, tag="rden", name=f"rden{h}_{ich}")
                    nc.vector.reciprocal_approx_fast(rden[:], den[:])
                    rbc = sb_nrm.tile([64, 512], f32, tag="rbc", name=f"rbc{h}_{ich}")
                    nc.gpsimd.partition_broadcast(rbc[:], rden[:])
                    with nc.allow_low_precision(reason="bf16 normalized attn out"):
                        if s == 0:
                            nc.vector.tensor_mul(o_sb[0:64, pair, isl],
                                                 ops[s][0:64, :], rbc[:])
                        else:
                            # odd head lands on partitions 64-127: DVE can't
                            # shift partitions, so normalize into a staging
                            # tile and SBUF->SBUF DMA it across
                            ot = sb_nrm.tile([64, 512], bf16, tag="ot",
                                             name=f"ot{h}_{ich}")
                            nc.vector.tensor_mul(ot[:], ops[s][0:64, :], rbc[:])
                            nc.sync.dma_start(out=o_sb[64:128, pair, isl], in_=ot[:])

            def proj_tile(ib, ec):
                """y_part rows [ib*128,+128), cols [ec*512,+512): two K=128
                head-pair matmuls; bias added at the eviction."""
                ibs = slice(ib * 128, (ib + 1) * 128)
                yps = ps_big.tile([128, 512], f32, tag="big", name=f"yps{ib}_{ec}")
                for p in range(2):
                    nc.tensor.matmul(yps[:], o_sb[:, p, ibs],
                                     wo_sb[:, p, ec * 512:(ec + 1) * 512],
                                     start=(p == 0), stop=(p == 1))
                ysb = sb_work.tile([128, 512], bf16, tag="y", name=f"ysb{ib}_{ec}")
                with nc.allow_low_precision(reason="bf16 partials for the reduce-scatter"):
                    nc.vector.tensor_add(ysb[:], yps[:],
                                         bias_bc[:, ec * 512:(ec + 1) * 512])
                nc.sync.dma_start(out=y_part[ibs, ec * 512:(ec + 1) * 512], in_=ysb[:])

            def emit_rs(row0, nrows, orow0):
                nc.gpsimd.collective_compute(
                    "ReduceScatter",
                    mybir.AluOpType.add,
                    replica_groups=groups,
                    ins=[y_part[row0:row0 + nrows, :]],
                    outs=[y_red[orow0:orow0 + nrows // GSIZE, :]],
                )

            # block order: pairs together so each ich's projection frees early
            blocks = [(0, 0), (0, 1), (1, 0), (1, 1), (2, 0), (2, 1), (3, 0), (3, 1)]

            def proj_inserts(ich, rs_chunks):
                """spread the 8 proj tiles over 4 slots; RS after the last tile"""
                ins = {}
                for k in range(4):
                    ib = ich * 4 + k
                    ins[k] = [lambda a=ib: proj_tile(a, 0), lambda a=ib: proj_tile(a, 1)]
                ins[3] += [lambda c=c: emit_rs(*c) for c in rs_chunks]
                return ins

            for idx, (ich, pair) in enumerate(blocks):
                if idx == 0:
                    # fuse the projection phase into block 0: per q-chunk,
                    # emit its qkT/v chains just before the attention groups
                    # that need them; q23/k23 chains run as fillers after
                    # each attnV window (avb=1 -> 8 slots)
                    pre = {}
                    ins = {}
                    for q in range(4):
                        pre[2 * q] = [lambda a=q: qkT_chain(2, a),
                                      lambda a=q: qkT_chain(0, a),
                                      lambda a=4 * q: v_chain(a),
                                      lambda a=4 * q + 1: v_chain(a)]
                        pre[2 * q + 1] = [lambda a=4 * q + 2: v_chain(a),
                                          lambda a=4 * q + 3: v_chain(a)]
                        ins[2 * q] = [lambda a=q: qkT_chain(1, a)]
                        ins[2 * q + 1] = [lambda a=q: qkT_chain(3, a)]
                    attention_block(ich, pair, 1, ins, pre_g=pre)
                    s1.close()  # xT / proj weights no longer needed
                elif idx in (2, 4, 6):
                    pich = idx // 2 - 1  # previous ich, fully normalized
                    attention_block(ich, pair, 2, proj_inserts(pich, [CHUNKS[pich]]))
                else:
                    attention_block(ich, pair, 2, {})

            # tail: last chunk's projection, split so the final RS is 256 rows
            for ib in (12, 13):
                proj_tile(ib, 0)
                proj_tile(ib, 1)
            emit_rs(*CHUNKS[3])
            for ib in (14, 15):
                proj_tile(ib, 0)
                proj_tile(ib, 1)
            emit_rs(*CHUNKS[4])
            # collectives may not write IO tensors: bounce DRAM->DRAM at the
            # very end, where the RS-completion waits cannot head-of-line
            # block anything else on the queue
            for (row0, nrows, orow0) in CHUNKS:
                nc.sync.dma_start(out=y_out[orow0:orow0 + nrows // GSIZE, :],
                                  in_=y_red[orow0:orow0 + nrows // GSIZE, :])

    nc.finalize()
    return nc


def _make_in_maps(x, w_qkv, w_out, b_out):
    import ml_dtypes

    bf16 = ml_dtypes.bfloat16
    x = np.asarray(x, dtype=np.float32)
    w_qkv = np.asarray(w_qkv, dtype=np.float32)
    w_out = np.asarray(w_out, dtype=np.float32)
    b_out = np.asarray(b_out, dtype=np.float32)
    zeros_bias = np.zeros((1, D), dtype=np.float32)
    in_maps = []
    for c in range(NCORES):
        b = c // GSIZE
        h0 = (c % GSIZE) * HPC
        cols = np.arange(h0 * DH, (h0 + HPC) * DH)
        wq = w_qkv[:, cols]
        wk = w_qkv[:, D + cols]
        wv = w_qkv[:, 2 * D + cols]
        xT = np.ascontiguousarray(x[b].T)  # [D, N]
        # pack as [q, kb, 128, 512] so each (q, kb) chunk is contiguous
        xT4 = xT.reshape(KB, 128, 4, 512).transpose(2, 0, 1, 3).reshape(4 * KB * 128, 512)
        in_maps.append({
            "xT4": np.ascontiguousarray(xT4).astype(bf16),
            "wqk": np.ascontiguousarray(np.concatenate([wq, wk], axis=1)).astype(bf16),
            "wv": np.ascontiguousarray(wv).astype(bf16),
            "wout": np.ascontiguousarray(w_out[cols, :]).astype(bf16),
            "bias": (b_out[None, :] if c % GSIZE == 0 else zeros_bias).astype(bf16),
        })
    return in_maps


def _assemble(results, x_shape):
    B = x_shape[0]
    y = np.empty((B, N, D), dtype=np.float32)
    for b in range(B):
        for g in range(GSIZE):
            shard = np.asarray(results[b * GSIZE + g]["y"], dtype=np.float32)  # [512, D]
            for (row0, nrows, orow0) in CHUNKS:
                n4 = nrows // GSIZE
                y[b, row0 + g * n4: row0 + (g + 1) * n4, :] = \
                    shard[orow0:orow0 + n4, :]
    return y


def kernel(x, w_qkv, w_out, b_out):
    from concourse.bass_utils import run_bass_kernel_spmd

    if "nc" not in _cached:
        _cached["nc"] = _build_nc()
    nc = _cached["nc"]
    in_maps = _make_in_maps(x, w_qkv, w_out, b_out)
    res = run_bass_kernel_spmd(nc, in_maps, list(range(NCORES)))
    return _assemble(res.results, np.asarray(x).shape)
